# revision 20
# baseline (speedup 1.0000x reference)
"""Gaussian blur 101x101 (separable) on 4096x4096 fp32, 8 NeuronCores.

Strategy: the 2D conv kernel W = outer(gv, gh) is rank-1, so the blur is two
1D 101-tap convs. Rows are sharded 512/core; each core gets a host-prepared
padded strip (50-row halo, zero-padded edges, plus 50/78 zero columns) so the
on-device program is uniform across cores with no collectives.

Each 1D conv maps onto the PE array as banded matmuls with 128-row
contraction windows:
  pass1: tmpT[j', i] = sum_r x[r, j'] gv[r - i + 50]
         matmul(lhsT = x[rows win, cols 128a:+128], rhs = Gv_d) -> PSUM
  pass2: out[i, j] = sum_j' tmpT[j', i] gh[j' - j + 50]
         matmul(lhsT = tmpT[win a][:, 128c:+128], rhs = Gh_d) -> PSUM
with shared band tiles G_d[k, f] = g[k - f + d], d in {0, 128, 256},
f-chunks of 256 (float32r runs 1 cycle/row at moving dim >= 256).
tmpT tiles are stored at the 128-row windows pass2 needs (offset -50), so no
transposes or partition-shifts are required anywhere.
"""

import os
from contextlib import ExitStack

import numpy as np

import concourse.bass as bass  # noqa: F401  (AP types come via tile/bacc)
import concourse.mybir as mybir
import concourse.tile as tile
from concourse import bacc, bass_utils

H = 4096
W = 4096
TAPS = 101
PAD = 50
N_CORES = 8
RPC = H // N_CORES          # 512 output rows per core
NW1 = 5                     # input row windows of 128 per core
XP_ROWS = 128 * NW1         # 640 = 512 + 100 halo + 28 slack (zeros)
NA = 33                     # tmpT column windows of 128
XP_COLS = 128 * NA          # 4224 = 50 + 4096 + 78 (zero-padded cols)
FB = 256                    # band free width per matmul
DT = mybir.dt.float32

_compiled = {}


class _FastExitTC(tile.TileContext):
    """TileContext whose exit skips the per-semaphore clear storm.

    The stock exit emits dma_reset + sem_clear for every allocated semaphore
    (~250 here) plus a second all-engine barrier — ~8us of pure tail on a
    NEFF that is loaded, executed once, and unloaded. The drain + one
    barrier (which gate output-DMA completion) are kept.
    """

    def _drain_and_barrier(self, tick_clock, wait_clock):
        from concourse.vector_clock import ScopedClock

        drain_inst = self.nc.sync.drain()
        wait_clock.add_sem_waits(
            drain_inst.ins, ScopedClock({None: tick_clock.global_clock})
        )
        self.nc.all_engine_barrier()
        popped = self.nc._tile_sem_poison_stack.pop()
        assert popped is self._sem_poison


def _build_nc(mm_dtype):
    nc = bacc.Bacc(
        "TRN2",
        target_bir_lowering=False,
        debug=False,
        enable_asserts=False,
        num_devices=N_CORES,
    )
    xp = nc.dram_tensor("xp", [XP_ROWS, XP_COLS], mm_dtype, kind="ExternalInput").ap()
    bandsV = nc.dram_tensor(
        "bandsV", [128, 3 * FB], mm_dtype, kind="ExternalInput"
    ).ap()
    bandsH = nc.dram_tensor(
        "bandsH", [128, 3 * FB], mm_dtype, kind="ExternalInput"
    ).ap()
    y = nc.dram_tensor("y", [RPC, W], DT, kind="ExternalOutput").ap()

    with _FastExitTC(nc) as tc, ExitStack() as ctx:
        xw_pool = ctx.enter_context(tc.tile_pool(name="xw", bufs=1))
        band_pool = ctx.enter_context(tc.tile_pool(name="bands", bufs=1))
        tm_pool = ctx.enter_context(tc.tile_pool(name="tm", bufs=1))
        p1_pool = ctx.enter_context(tc.tile_pool(name="p1", bufs=3, space="PSUM"))
        warm_pool = ctx.enter_context(tc.tile_pool(name="warm", bufs=1, space="PSUM"))
        p2_pool = ctx.enter_context(tc.tile_pool(name="p2", bufs=4, space="PSUM"))
        st_pool = ctx.enter_context(tc.tile_pool(name="st", bufs=6))

        # column-chunked window loads so pass1's first tiles aren't gated on
        # full 2.2MB window transfers; chunk order matches pass1's a-order
        ccuts = [0, 256, 1280, 2304, 3328, XP_COLS]
        xw = [
            xw_pool.tile([128, XP_COLS], mm_dtype, tag=f"xw{w}", name=f"xw{w}")
            for w in range(NW1)
        ]

        # spread DMA issue over three queues — a single queue only issues one
        # descriptor-gen op per ~600ns, which starves the PE at kernel start
        dma_engines = [nc.sync, nc.scalar]
        bv = band_pool.tile([128, 3 * FB], mm_dtype, tag="bv")
        nc.sync.dma_start(bv[:, 0:FB], bandsV[:, 0:FB])
        bh = band_pool.tile([128, 3 * FB], mm_dtype, tag="bh")
        nc.scalar.dma_start(bh[:], bandsH[:])

        # PE warmup: junk matmuls on the band tile while input streams in, so
        # HAM reaches K=8/8 around when the real matmuls start
        wps = warm_pool.tile([128, FB], DT, name="wps")
        for _ in range(16):
            nc.tensor.matmul(
                wps[:], lhsT=bv[:, 0:128], rhs=bv[:, 0:FB], start=True, stop=True
            )
        k = 0
        for ci in range(len(ccuts) - 1):
            cs, ce = ccuts[ci], ccuts[ci + 1]
            for w in range(NW1):
                eng = dma_engines[k % 2]
                k += 1
                eng.dma_start(xw[w][:, cs:ce], xp[128 * w : 128 * (w + 1), cs:ce])
            if ci == 0:
                nc.sync.dma_start(bv[:, FB:], bandsV[:, FB:])

        # pass 1: tmpT windows, each [128 j', 512 i]; stored as mm_dtype so the
        # PSUM evacuation copy performs the FP32r rounding the verifier wants
        tm = []
        for a in range(NA):
            ps1 = p1_pool.tile([128, RPC], DT)
            for b in range(2):
                for di in range(3):
                    w = 2 * b + di
                    nc.tensor.matmul(
                        ps1[:, FB * b : FB * (b + 1)],
                        lhsT=xw[w][:, 128 * a : 128 * (a + 1)],
                        rhs=bv[:, FB * di : FB * (di + 1)],
                        start=(di == 0),
                        stop=(di == 2),
                    )
            tma = tm_pool.tile([128, RPC], mm_dtype, tag=f"tm{a}", name=f"tm{a}")
            nc.vector.tensor_copy(tma[:], ps1[:])
            tm.append(tma)

        # pass 2: out tiles [128 i, 512 j]; t2-major so tiles unlock in the
        # order pass1 produces tm windows, letting the scheduler backfill PE
        # while pass1 streams input. Evacuations go to the otherwise-idle ACT;
        # output DMAs issue immediately, alternating queues.
        for t2 in range(W // (2 * FB)):
            for cpt in range(RPC // 128):
                ps2 = p2_pool.tile([128, 2 * FB], DT)
                for hf in range(2):
                    b2 = 2 * t2 + hf
                    for ai in range(3):
                        a = 2 * b2 + ai
                        nc.tensor.matmul(
                            ps2[:, FB * hf : FB * (hf + 1)],
                            lhsT=tm[a][:, 128 * cpt : 128 * (cpt + 1)],
                            rhs=bh[:, FB * ai : FB * (ai + 1)],
                            start=(ai == 0),
                            stop=(ai == 2),
                        )
                st = st_pool.tile([128, 2 * FB], DT, name=f"st_{t2}_{cpt}", tag="st")
                nc.scalar.copy(st[:], ps2[:])
                eng = dma_engines[(t2 * 4 + cpt) % 2]
                eng.dma_start(
                    y[128 * cpt : 128 * (cpt + 1), 512 * t2 : 512 * (t2 + 1)],
                    st[:],
                )

    nc.compile()
    return nc


def _get_nc(mm_dtype):
    key = str(mm_dtype)
    if key not in _compiled:
        _compiled[key] = _build_nc(mm_dtype)
    return _compiled[key]


def _make_band(g, d):
    # G_d[k, f] = g[k - f + d], zero outside [0, TAPS)
    idx = np.arange(128)[:, None] - np.arange(FB)[None, :] + d
    valid = (idx >= 0) & (idx < TAPS)
    return np.where(valid, g[np.clip(idx, 0, TAPS - 1)], 0.0).astype(np.float32)


def kernel(x: np.ndarray, weight: np.ndarray) -> np.ndarray:
    x = np.asarray(x, dtype=np.float32)
    Wm = np.asarray(weight, dtype=np.float32).reshape(TAPS, TAPS)
    assert x.shape == (H, W), x.shape

    # rank-1 (separable) decomposition of the 2D kernel
    u, s, vt = np.linalg.svd(Wm.astype(np.float64))
    gv = (u[:, 0] * np.sqrt(s[0]))
    gh = (vt[0] * np.sqrt(s[0]))
    if gv.sum() < 0:
        gv, gh = -gv, -gh
    gv = gv.astype(np.float32)
    gh = gh.astype(np.float32)

    bandsV = np.concatenate([_make_band(gv, d) for d in (0, 128, 256)], axis=1)
    bandsH = np.concatenate([_make_band(gh, d) for d in (0, 128, 256)], axis=1)

    # padded per-core strips: rows [r0-50, r0+590), cols [-50, 4174), zeros
    # outside the image
    in_maps = []
    for c in range(N_CORES):
        r0 = c * RPC
        xp = np.zeros((XP_ROWS, XP_COLS), np.float32)
        lo = r0 - PAD
        hi = min(r0 + RPC + PAD, H)
        src_lo = max(lo, 0)
        xp[src_lo - lo : hi - lo, PAD : PAD + W] = x[src_lo:hi]
        in_maps.append({"xp": xp, "bandsV": bandsV, "bandsH": bandsH})

    mm_dtype = (
        mybir.dt.float32
        if os.environ.get("BLUR_MM_DTYPE") == "fp32"
        else mybir.dt.float32r
    )
    nc = _get_nc(mm_dtype)

    trace = os.environ.get("BLUR_TRACE") == "1"
    res = bass_utils.run_bass_kernel_spmd(
        nc, in_maps, core_ids=list(range(N_CORES)), trace=trace
    )
    if trace:
        print(f"HW exec time: {res.exec_time_ns} ns")
        print(f"mean exec time: {res.mean_exec_time_ns} ns")
        if res.instructions_and_trace is not None:
            print(f"trace: {res.instructions_and_trace[1]}")

    out = np.concatenate([res.results[c]["y"] for c in range(N_CORES)], axis=0)
    return out[None, None]


# revision 21
# speedup vs baseline: 1.0583x; 1.0583x over previous
"""Gaussian blur 101x101 (separable) on 4096x4096 fp32, 8 NeuronCores.

Strategy: the 2D conv kernel W = outer(gv, gh) is rank-1, so the blur is two
1D 101-tap convs. Rows are sharded 512/core; each core gets a host-prepared
padded strip (50-row halo, zero-padded edges, plus 50/78 zero columns) so the
on-device program is uniform across cores with no collectives.

Each 1D conv maps onto the PE array as banded matmuls with 128-row
contraction windows:
  pass1: tmpT[j', i] = sum_r x[r, j'] gv[r - i + 50]
         matmul(lhsT = x[rows win, cols 128a:+128], rhs = Gv_d) -> PSUM
  pass2: out[i, j] = sum_j' tmpT[j', i] gh[j' - j + 50]
         matmul(lhsT = tmpT[win a][:, 128c:+128], rhs = Gh_d) -> PSUM
with shared band tiles G_d[k, f] = g[k - f + d], d in {0, 128, 256},
f-chunks of 256 (float32r runs 1 cycle/row at moving dim >= 256).
tmpT tiles are stored at the 128-row windows pass2 needs (offset -50), so no
transposes or partition-shifts are required anywhere.
"""

import os
from contextlib import ExitStack

import numpy as np

import concourse.bass as bass  # noqa: F401  (AP types come via tile/bacc)
import concourse.mybir as mybir
import concourse.tile as tile
from concourse import bacc, bass_utils

H = 4096
W = 4096
TAPS = 101
PAD = 50
N_CORES = 8
RPC = H // N_CORES          # 512 output rows per core
NW1 = 5                     # input row windows of 128 per core
XP_ROWS = 128 * NW1         # 640 = 512 + 100 halo + 28 slack (zeros)
NA = 33                     # tmpT column windows of 128
XP_COLS = 128 * NA          # 4224 = 50 + 4096 + 78 (zero-padded cols)
FB = 256                    # band free width per matmul
DT = mybir.dt.float32

_compiled = {}


class _FastExitTC(tile.TileContext):
    """TileContext whose exit skips the per-semaphore clear storm.

    The stock exit emits dma_reset + sem_clear for every allocated semaphore
    (~250 here) plus a second all-engine barrier — ~8us of pure tail on a
    NEFF that is loaded, executed once, and unloaded. The drain + one
    barrier (which gate output-DMA completion) are kept.
    """

    def _drain_and_barrier(self, tick_clock, wait_clock):
        from concourse.vector_clock import ScopedClock

        drain_inst = self.nc.sync.drain()
        wait_clock.add_sem_waits(
            drain_inst.ins, ScopedClock({None: tick_clock.global_clock})
        )
        self.nc.all_engine_barrier()
        popped = self.nc._tile_sem_poison_stack.pop()
        assert popped is self._sem_poison


def _build_nc(mm_dtype):
    nc = bacc.Bacc(
        "TRN2",
        target_bir_lowering=False,
        debug=False,
        enable_asserts=False,
        num_devices=N_CORES,
    )
    xp = nc.dram_tensor("xp", [XP_ROWS, XP_COLS], mm_dtype, kind="ExternalInput").ap()
    bandsV = nc.dram_tensor(
        "bandsV", [128, 3 * FB], mm_dtype, kind="ExternalInput"
    ).ap()
    bandsH = nc.dram_tensor(
        "bandsH", [128, 3 * FB], mm_dtype, kind="ExternalInput"
    ).ap()
    y = nc.dram_tensor("y", [RPC, W], DT, kind="ExternalOutput").ap()

    with _FastExitTC(nc) as tc, ExitStack() as ctx:
        xw_pool = ctx.enter_context(tc.tile_pool(name="xw", bufs=1))
        band_pool = ctx.enter_context(tc.tile_pool(name="bands", bufs=1))
        tm_pool = ctx.enter_context(tc.tile_pool(name="tm", bufs=1))
        p1_pool = ctx.enter_context(tc.tile_pool(name="p1", bufs=3, space="PSUM"))
        warm_pool = ctx.enter_context(tc.tile_pool(name="warm", bufs=1, space="PSUM"))
        p2_pool = ctx.enter_context(tc.tile_pool(name="p2", bufs=4, space="PSUM"))
        st_pool = ctx.enter_context(tc.tile_pool(name="st", bufs=6))

        # column-chunked window loads so pass1's first tiles aren't gated on
        # full 2.2MB window transfers; chunk order matches pass1's a-order
        ccuts = [0, 256, 1280, 2304, 3328, XP_COLS]
        xw = [
            xw_pool.tile([128, XP_COLS], mm_dtype, tag=f"xw{w}", name=f"xw{w}")
            for w in range(NW1)
        ]

        # spread DMA issue over three queues — a single queue only issues one
        # descriptor-gen op per ~600ns, which starves the PE at kernel start
        dma_engines = [nc.sync, nc.scalar]
        bv = band_pool.tile([128, 3 * FB], mm_dtype, tag="bv")
        nc.sync.dma_start(bv[:], bandsV[:])
        bh = band_pool.tile([128, 3 * FB], mm_dtype, tag="bh")
        nc.scalar.dma_start(bh[:], bandsH[:])

        # PE warmup: junk matmuls on the band tile while input streams in, so
        # HAM reaches K=8/8 around when the real matmuls start
        wps = warm_pool.tile([128, FB], DT, name="wps")
        for _ in range(16):
            nc.tensor.matmul(
                wps[:], lhsT=bv[:, 0:128], rhs=bv[:, 0:FB], start=True, stop=True
            )
        k = 0
        for ci in range(len(ccuts) - 1):
            cs, ce = ccuts[ci], ccuts[ci + 1]
            for w in range(NW1):
                eng = dma_engines[k % 2]
                k += 1
                eng.dma_start(xw[w][:, cs:ce], xp[128 * w : 128 * (w + 1), cs:ce])

        # pass 1: tmpT windows, each [128 j', 512 i]; stored as mm_dtype so the
        # PSUM evacuation copy performs the FP32r rounding the verifier wants
        tm = []
        for a in range(NA):
            ps1 = p1_pool.tile([128, RPC], DT)
            for b in range(2):
                for di in range(3):
                    w = 2 * b + di
                    nc.tensor.matmul(
                        ps1[:, FB * b : FB * (b + 1)],
                        lhsT=xw[w][:, 128 * a : 128 * (a + 1)],
                        rhs=bv[:, FB * di : FB * (di + 1)],
                        start=(di == 0),
                        stop=(di == 2),
                    )
            tma = tm_pool.tile([128, RPC], mm_dtype, tag=f"tm{a}", name=f"tm{a}")
            nc.vector.tensor_copy(tma[:], ps1[:])
            tm.append(tma)

        # pass 2: out tiles [128 i, 512 j]; t2-major so tiles unlock in the
        # order pass1 produces tm windows, letting the scheduler backfill PE
        # while pass1 streams input. Evacuations go to the otherwise-idle ACT;
        # output DMAs issue immediately, alternating queues.
        for t2 in range(W // (2 * FB)):
            for cpt in range(RPC // 128):
                ps2 = p2_pool.tile([128, 2 * FB], DT)
                for hf in range(2):
                    b2 = 2 * t2 + hf
                    for ai in range(3):
                        a = 2 * b2 + ai
                        nc.tensor.matmul(
                            ps2[:, FB * hf : FB * (hf + 1)],
                            lhsT=tm[a][:, 128 * cpt : 128 * (cpt + 1)],
                            rhs=bh[:, FB * ai : FB * (ai + 1)],
                            start=(ai == 0),
                            stop=(ai == 2),
                        )
                st = st_pool.tile([128, 2 * FB], DT, name=f"st_{t2}_{cpt}", tag="st")
                nc.scalar.copy(st[:], ps2[:])
                eng = dma_engines[(t2 * 4 + cpt) % 2]
                eng.dma_start(
                    y[128 * cpt : 128 * (cpt + 1), 512 * t2 : 512 * (t2 + 1)],
                    st[:],
                )

    nc.compile()
    return nc


def _get_nc(mm_dtype):
    key = str(mm_dtype)
    if key not in _compiled:
        _compiled[key] = _build_nc(mm_dtype)
    return _compiled[key]


def _make_band(g, d):
    # G_d[k, f] = g[k - f + d], zero outside [0, TAPS)
    idx = np.arange(128)[:, None] - np.arange(FB)[None, :] + d
    valid = (idx >= 0) & (idx < TAPS)
    return np.where(valid, g[np.clip(idx, 0, TAPS - 1)], 0.0).astype(np.float32)


def kernel(x: np.ndarray, weight: np.ndarray) -> np.ndarray:
    x = np.asarray(x, dtype=np.float32)
    Wm = np.asarray(weight, dtype=np.float32).reshape(TAPS, TAPS)
    assert x.shape == (H, W), x.shape

    # rank-1 (separable) decomposition of the 2D kernel
    u, s, vt = np.linalg.svd(Wm.astype(np.float64))
    gv = (u[:, 0] * np.sqrt(s[0]))
    gh = (vt[0] * np.sqrt(s[0]))
    if gv.sum() < 0:
        gv, gh = -gv, -gh
    gv = gv.astype(np.float32)
    gh = gh.astype(np.float32)

    bandsV = np.concatenate([_make_band(gv, d) for d in (0, 128, 256)], axis=1)
    bandsH = np.concatenate([_make_band(gh, d) for d in (0, 128, 256)], axis=1)

    # padded per-core strips: rows [r0-50, r0+590), cols [-50, 4174), zeros
    # outside the image
    in_maps = []
    for c in range(N_CORES):
        r0 = c * RPC
        xp = np.zeros((XP_ROWS, XP_COLS), np.float32)
        lo = r0 - PAD
        hi = min(r0 + RPC + PAD, H)
        src_lo = max(lo, 0)
        xp[src_lo - lo : hi - lo, PAD : PAD + W] = x[src_lo:hi]
        in_maps.append({"xp": xp, "bandsV": bandsV, "bandsH": bandsH})

    mm_dtype = (
        mybir.dt.float32
        if os.environ.get("BLUR_MM_DTYPE") == "fp32"
        else mybir.dt.float32r
    )
    nc = _get_nc(mm_dtype)

    trace = os.environ.get("BLUR_TRACE") == "1"
    res = bass_utils.run_bass_kernel_spmd(
        nc, in_maps, core_ids=list(range(N_CORES)), trace=trace
    )
    if trace:
        print(f"HW exec time: {res.exec_time_ns} ns")
        print(f"mean exec time: {res.mean_exec_time_ns} ns")
        if res.instructions_and_trace is not None:
            print(f"trace: {res.instructions_and_trace[1]}")

    out = np.concatenate([res.results[c]["y"] for c in range(N_CORES)], axis=0)
    return out[None, None]
